# revision 2
# baseline (speedup 1.0000x reference)
"""GATv2 layer on 8 Trainium2 NeuronCores (Bass/Tile SPMD).

Identity: exp(lrelu(z)) = max(exp z, exp 0.2z); dividing the softmax by
exp(0.2 ei) gives the unnormalized masked score
    pm[j,i] = adj[j,i] * max(Ti[i]*F[j], V[j])
with Ti = exp(0.8 ei), F = exp(ej), V = exp(0.2 ej).

Aggregation per head: acc = sum_j [0.25*h | 1][j] * pm[j,i]  (PE, [65,512]).

The elementwise pm production is split across DVE and ACT so both engines
work concurrently (chunk pairs, interleaved D/A blocks):
  D pair:  q  = max(tib*F, V)       tensor_scalar 4x   (DVE)
           pm = q * adj             tensor_tensor 2x, batched (DVE)
  A pair:  aq = relu(tib*F - V)     ACT (free per-partition scale/bias)
           m1 = aq * adj            tensor_tensor 2x, batched (DVE)
           + extra PE chain  acc += rhs2^T @ adj  with rhs2 = V*[0.25h | 1]
           (so pm = m1 + V*adj is accumulated in two parts)

Host precomputes h, Ti, F, V, rhs, rhs2 (all O(N*F^2)); the device does all
O(N^2) work.  Core c owns destination rows i in [512c, 512c+512); adj is fed
pre-transposed+sliced per core, fp16, so scores build directly in
[j_partition, i_free] layout for the PE aggregation matmul.
"""

import contextlib
import os
import sys

import numpy as np

for _p in ("/opt/trn_rl_repo", "/root/.axon_site/_ro/trn_rl_repo"):
    if os.path.isdir(_p) and _p not in sys.path:
        sys.path.append(_p)

import concourse.bass as bass
import concourse.mybir as mybir
from concourse import bacc
import concourse.tile as tile
from concourse.tile import add_dep_helper
from concourse.bass_utils import run_bass_kernel_spmd

N = 4096
F_IN = 256
HEADS = 4
F_OUT = 64
CORES = 8
I_PER_CORE = N // CORES          # 512
P = 128
NJC = N // P                     # 32 j-chunks
NPAIR = NJC // 2                 # 16 chunk pairs
NIC = I_PER_CORE // P            # 4 i-chunks
ICOL = F_OUT + 1                 # 65: [0.25*h | ones] per head
A_PAIRS = 9                      # chunk pairs routed to the ACT path

F32 = mybir.dt.float32
F16 = mybir.dt.float16

_BASS = None
LAST_RESULT = None


def _pair_kinds():
    """A pairs at odd slots (strict alternation), D at both ends; the 8th A
    (if any) goes to slot 14."""
    kinds = ["D"] * NPAIR
    odd = [1, 3, 5, 7, 9, 11, 13]
    extra = [14, 2, 6, 10, 4, 8, 12, 0]
    slots = (odd + extra)[:A_PAIRS]
    for s in slots:
        kinds[s] = "A"
    return kinds


def _build(reps=1):
    nc = bacc.Bacc()
    rhs_d = nc.dram_tensor("rhs", [P, NJC, HEADS, ICOL], F16, kind="ExternalInput")
    NA = 2 * A_PAIRS
    rhs2_d = nc.dram_tensor("rhs2", [P, max(NA, 1), 2, P], F16,
                            kind="ExternalInput")
    vq4_d = nc.dram_tensor("vq4", [P, max(NA, 1), HEADS], F16,
                           kind="ExternalInput")
    tib_d = nc.dram_tensor("tib", [P, HEADS, I_PER_CORE], F16, kind="ExternalInput")
    fj_d = nc.dram_tensor("fj", [P, NJC, HEADS], F32, kind="ExternalInput")
    vq_d = nc.dram_tensor("vq", [P, NJC, HEADS], F32, kind="ExternalInput")
    nvq_d = nc.dram_tensor("nvq", [P, NJC, HEADS], F32, kind="ExternalInput")
    adjT_d = nc.dram_tensor("adjT", [N, I_PER_CORE], F16, kind="ExternalInput")
    # per-head [num | den] accumulators; the host does divide + head-mean
    out_d = nc.dram_tensor("out", [ICOL, HEADS, I_PER_CORE], F32,
                           kind="ExternalOutput")
    out2_d = nc.dram_tensor("out2", [P, 2, I_PER_CORE], F32,
                            kind="ExternalOutput")
    outd_d = nc.dram_tensor("outd", [HEADS, I_PER_CORE], F32,
                            kind="ExternalOutput")

    RELU = mybir.ActivationFunctionType.Relu
    CPY = mybir.ActivationFunctionType.Copy
    MULT = mybir.AluOpType.mult
    MAX = mybir.AluOpType.max
    ADD = mybir.AluOpType.add

    kinds = _pair_kinds()

    with tile.TileContext(nc) as tc:
        with (
            tc.tile_pool(name="cst", bufs=1) as cst,
            tc.tile_pool(name="adj", bufs=1) as adjp,
            tc.tile_pool(name="qpm", bufs=3) as qpm,
            tc.tile_pool(name="aqp", bufs=4) as aqp,
            tc.tile_pool(name="fin", bufs=1) as fin,
            tc.tile_pool(name="psacc", bufs=1, space="PSUM") as psacc,
            (tc.For_i(0, reps, 1) if reps > 1 else contextlib.nullcontext()),
        ):
            # ---- loads -------------------------------------------------
            # order: small vectors, first adj chunks, rhs (needed by first
            # MMs), then the rest of adj interleaved with rhs2
            tib = cst.tile([P, HEADS, I_PER_CORE], F16, tag="tib")
            nc.sync.dma_start(tib[:, :2], tib_d[:, :2])
            nc.sync.dma_start(tib[:, 2:], tib_d[:, 2:])
            fj = cst.tile([P, NJC, HEADS], F32, tag="fj")
            nc.sync.dma_start(fj[:], fj_d[:])
            vq = cst.tile([P, NJC, HEADS], F32, tag="vq")
            nc.sync.dma_start(vq[:], vq_d[:])
            nvq = cst.tile([P, NJC, HEADS], F32, tag="nvq")
            nc.sync.dma_start(nvq[:], nvq_d[:])
            adj_sb = adjp.tile([P, NJC, I_PER_CORE], F16, tag="adj_sb")
            rhs = cst.tile([P, NJC, HEADS, ICOL], F16, tag="rhs")
            rhs2 = cst.tile([P, max(NA, 1), 2, P], F16, tag="rhs2")
            vq4 = cst.tile([P, max(NA, 1), HEADS], F16, tag="vq4")
            nc.sync.dma_start(vq4[:], vq4_d[:])
            a_slot = {}
            for pi in range(NPAIR):
                if kinds[pi] == "A":
                    for jj in range(2):
                        a_slot[2 * pi + jj] = len(a_slot)
            # interleave: rhs/rhs2 quarters between adj chunk groups so the
            # first pairs' operands all arrive early
            if A_PAIRS:
                nc.sync.dma_start(rhs2[:, :NA // 2], rhs2_d[:, :NA // 2])
            nc.sync.dma_start(rhs[:, :8], rhs_d[:, :8])
            for j in range(6):
                nc.sync.dma_start(adj_sb[:, j, :], adjT_d[j * P:(j + 1) * P, :])
            nc.sync.dma_start(rhs[:, 8:20], rhs_d[:, 8:20])
            for j in range(6, 10):
                nc.sync.dma_start(adj_sb[:, j, :], adjT_d[j * P:(j + 1) * P, :])
            if A_PAIRS:
                nc.sync.dma_start(rhs2[:, NA // 2:], rhs2_d[:, NA // 2:])
            for j in range(10, 14):
                nc.sync.dma_start(adj_sb[:, j, :], adjT_d[j * P:(j + 1) * P, :])
            nc.sync.dma_start(rhs[:, 20:], rhs_d[:, 20:])
            for j in range(14, 18):
                nc.sync.dma_start(adj_sb[:, j, :], adjT_d[j * P:(j + 1) * P, :])
            for j in range(18, NJC):
                nc.sync.dma_start(adj_sb[:, j, :], adjT_d[j * P:(j + 1) * P, :])

            # warm the ACT table set (Relu) at t=0 so the first real
            # activation doesn't pay the ~2.7us table load
            warm = fin.tile([P, 1], F32, tag="warm")
            nc.gpsimd.memset(warm[:], 0.0)
            nc.scalar.activation(warm[:], warm[:], RELU)

            # pre-touch operands so the first DVE/ACT ops need at most one
            # sync wait (HW encoding limit)
            junk = fin.tile([P, 4], F32, tag="junk")
            pt1 = nc.vector.tensor_copy(junk[:, 0:1], tib[:, 0, 0:1])
            pt2 = nc.vector.tensor_copy(junk[:, 1:2], fj[:, 0, 0:1])
            pt3 = nc.vector.tensor_copy(junk[:, 2:3], vq[:, 0, 0:1])
            pt4 = nc.vector.tensor_copy(junk[:, 3:4], nvq[:, 0, 0:1])
            pretouch = [pt1, pt2, pt3, pt4]

            acc = [psacc.tile([ICOL, I_PER_CORE], F32, name=f"acc{h}",
                              tag=f"acc{h}") for h in range(HEADS)]
            acc2 = [psacc.tile([P, I_PER_CORE], F32, name=f"acc2{g}",
                               tag=f"acc2{g}") for g in range(2)]
            accd = psacc.tile([HEADS, I_PER_CORE], F32, tag="accd")

            # per-head accumulation-group bookkeeping
            n_mms = {h: 0 for h in range(HEADS)}
            total_mms = NJC

            def mm(h, lhsT, rhs_op):
                first = n_mms[h] == 0
                n_mms[h] += 1
                last = n_mms[h] == total_mms
                nc.tensor.matmul(acc[h][:], lhsT, rhs_op,
                                 start=first, stop=last)

            first_dve = [True]

            def dep(qi):
                if first_dve[0]:
                    first_dve[0] = False
                    for pt in pretouch:
                        add_dep_helper(qi.ins, pt.ins, sync=False,
                                       reason="pretouch order")

            # The adj-chain MMs depend only on DMA — issue them all
            # upfront so the PE has work during the DVE/ACT ramp; their
            # accumulators retire mid-kernel and evacuate early.  Numerators
            # for head pairs (2g, 2g+1) are packed into one M=128 matmul;
            # denominators for all 4 heads into one M=4 matmul.
            a_chunk_list = [2 * pi + jj for pi in range(NPAIR)
                            if kinds[pi] == "A" for jj in range(2)]
            for n_, j in enumerate(a_chunk_list):
                s = a_slot[j]
                st = n_ == 0
                sp = n_ == len(a_chunk_list) - 1
                for g in range(2):
                    nc.tensor.matmul(acc2[g][:], rhs2[:, s, g, :],
                                     adj_sb[:, j, :], start=st, stop=sp)
                nc.tensor.matmul(accd[:], vq4[:, s, :], adj_sb[:, j, :],
                                 start=st, stop=sp)
            if a_chunk_list:
                out2sb = fin.tile([P, 2, I_PER_CORE], F32, tag="out2sb")
                outdsb = fin.tile([HEADS, I_PER_CORE], F32, tag="outdsb")
                for g in range(2):
                    nc.vector.tensor_copy(out2sb[:, g, :], acc2[g][:])
                nc.vector.tensor_copy(outdsb[:], accd[:])
                nc.sync.dma_start(out2_d[:], out2sb[:])
                nc.sync.dma_start(outd_d[:], outdsb[:])
            else:
                out2sb = fin.tile([P, 2, I_PER_CORE], F32, tag="out2sb")
                outdsb = fin.tile([HEADS, I_PER_CORE], F32, tag="outdsb")
                nc.vector.memset(out2sb[:], 0.0)
                nc.vector.memset(outdsb[:], 0.0)
                nc.sync.dma_start(out2_d[:], out2sb[:])
                nc.sync.dma_start(outd_d[:], outdsb[:])

            # software pipeline: A-pair ACT work is issued LOOKAHEAD pairs
            # early so the DVE never stalls at an A-pair's mask multiply
            # waiting on the ACT engine (DVE queues are strict FIFO).
            LOOKAHEAD = 3
            aq_tiles = {}

            def issue_act(pi):
                if pi >= NPAIR or kinds[pi] != "A":
                    return
                c0 = 2 * pi
                aq = aqp.tile([P, 2, HEADS, I_PER_CORE], F16, tag="aq")
                for jj in range(2):
                    j = c0 + jj
                    for h in range(HEADS):
                        qi = nc.scalar.activation(
                            aq[:, jj, h, :], tib[:, h, :], RELU,
                            scale=fj[:, j, h:h + 1],
                            bias=nvq[:, j, h:h + 1])
                        dep(qi)
                aq_tiles[pi] = aq

            for pi in range(LOOKAHEAD):
                issue_act(pi)

            for pi in range(NPAIR):
                issue_act(pi + LOOKAHEAD)
                c0 = 2 * pi
                adj_b = adj_sb[:, c0:c0 + 2, :].unsqueeze(2).broadcast_to(
                    [P, 2, HEADS, I_PER_CORE])
                if kinds[pi] == "D" and pi == NPAIR - 1:
                    # final pair: chunk 30 gets a per-chunk TT; chunk 31 goes
                    # per-head so the very last TT->MM bubble is minimal
                    j = c0
                    q4 = qpm.tile([P, HEADS, I_PER_CORE], F16, tag="q4s")
                    pm4 = qpm.tile([P, HEADS, I_PER_CORE], F16, tag="pm4s")
                    for h in range(HEADS):
                        qi = nc.vector.tensor_scalar(
                            q4[:, h, :], tib[:, h, :],
                            fj[:, j, h:h + 1], vq[:, j, h:h + 1],
                            op0=MULT, op1=MAX)
                        dep(qi)
                    adj_c = adj_sb[:, j, :].unsqueeze(1).broadcast_to(
                        [P, HEADS, I_PER_CORE])
                    nc.vector.tensor_tensor(pm4[:], q4[:], adj_c, op=MULT)
                    for h in range(HEADS):
                        mm(h, rhs[:, j, h, :], pm4[:, h, :])
                    j = c0 + 1
                    q4b = qpm.tile([P, HEADS, I_PER_CORE], F16, tag="q4s")
                    pm4b = qpm.tile([P, HEADS, I_PER_CORE], F16, tag="pm4s")
                    for h in range(HEADS):
                        nc.vector.tensor_scalar(
                            q4b[:, h, :], tib[:, h, :],
                            fj[:, j, h:h + 1], vq[:, j, h:h + 1],
                            op0=MULT, op1=MAX)
                    for h in range(HEADS):
                        nc.vector.tensor_tensor(
                            pm4b[:, h, :], q4b[:, h, :], adj_sb[:, j, :],
                            op=MULT)
                        mm(h, rhs[:, j, h, :], pm4b[:, h, :])
                elif kinds[pi] == "D":
                    q4 = qpm.tile([P, 2, HEADS, I_PER_CORE], F16, tag="q4")
                    pm4 = qpm.tile([P, 2, HEADS, I_PER_CORE], F16, tag="pm4")
                    for jj in range(2):
                        j = c0 + jj
                        for h in range(HEADS):
                            qi = nc.vector.tensor_scalar(
                                q4[:, jj, h, :], tib[:, h, :],
                                fj[:, j, h:h + 1], vq[:, j, h:h + 1],
                                op0=MULT, op1=MAX)
                            dep(qi)
                    nc.vector.tensor_tensor(pm4[:], q4[:], adj_b, op=MULT)
                    for jj in range(2):
                        j = c0 + jj
                        for h in range(HEADS):
                            mm(h, rhs[:, j, h, :], pm4[:, jj, h, :])
                else:
                    aq = aq_tiles.pop(pi)
                    m1 = qpm.tile([P, 2, HEADS, I_PER_CORE], F16, tag="m1")
                    nc.vector.tensor_tensor(m1[:], aq[:], adj_b, op=MULT)
                    for jj in range(2):
                        j = c0 + jj
                        for h in range(HEADS):
                            mm(h, rhs[:, j, h, :], m1[:, jj, h, :])

            # finalize: evacuate the four [num | den] accumulators into one
            # tile and ship with a single DMA; the host does divide +
            # head-mean (O(N*F) numpy)
            numt = fin.tile([ICOL, HEADS, I_PER_CORE], F32, tag="numt")
            nc.scalar.copy(numt[:, 0, :], acc[0][:])
            nc.vector.tensor_copy(numt[:, 1, :], acc[1][:])
            nc.sync.dma_start(out_d[:, :2], numt[:, :2])
            nc.scalar.copy(numt[:, 2, :], acc[2][:])
            nc.vector.tensor_copy(numt[:, 3, :], acc[3][:])
            nc.sync.dma_start(out_d[:, 2:], numt[:, 2:])

    nc.finalize()
    return nc


def _host_prep(x, adj, W, a):
    x = np.asarray(x, np.float32)
    adj = np.asarray(adj)
    W = np.asarray(W, np.float32)
    a = np.asarray(a, np.float32).reshape(-1)
    a1, a2 = a[:F_OUT], a[F_OUT:]

    h = (x @ W).reshape(N, HEADS, F_OUT)          # [N, H, F] f32
    ei = np.einsum("nhf,f->nh", h, a1)            # [N, H]
    ej = np.einsum("nhf,f->nh", h, a2)            # [N, H]

    ti16 = np.exp(0.8 * ei).astype(np.float16)    # [N, H]
    fjv = np.exp(ej).astype(np.float32)           # Sj*Vq = exp(ej)
    vqv = np.exp(0.2 * ej).astype(np.float32)

    hq = (0.25 * h).astype(np.float32)
    rhs = np.empty((P, NJC, HEADS, ICOL), np.float32)
    rhs[:, :, :, :F_OUT] = hq.reshape(NJC, P, HEADS, F_OUT).transpose(1, 0, 2, 3)
    rhs[:, :, :, F_OUT] = 1.0
    rhs2f = rhs * vqv.reshape(NJC, P, HEADS, 1).transpose(1, 0, 2, 3)
    kinds = _pair_kinds()
    a_chunks = [2 * pi + jj for pi in range(NPAIR) if kinds[pi] == "A"
                for jj in range(2)]
    na = max(len(a_chunks), 1)
    rhs2 = np.zeros((P, na, 2, P), np.float32)
    vq4 = np.zeros((P, na, HEADS), np.float32)
    for s, j in enumerate(a_chunks):
        for g in range(2):
            rhs2[:, s, g, :F_OUT] = rhs2f[:, j, 2 * g, :F_OUT]
            rhs2[:, s, g, F_OUT:2 * F_OUT] = rhs2f[:, j, 2 * g + 1, :F_OUT]
        vq4[:, s, :] = rhs2f[:, j, :, F_OUT]

    def jvec(v):
        return np.ascontiguousarray(v.reshape(NJC, P, HEADS).transpose(1, 0, 2))

    fj = jvec(fjv)
    vq = jvec(vqv)
    adjT = adj.T.astype(np.float16)               # [j, i]

    in_maps = []
    for c in range(CORES):
        sl = slice(c * I_PER_CORE, (c + 1) * I_PER_CORE)
        in_maps.append({
            "rhs": rhs.astype(np.float16),
            "rhs2": rhs2.astype(np.float16),
            "vq4": vq4.astype(np.float16),
            "tib": np.ascontiguousarray(
                np.broadcast_to(ti16[sl].T[None], (P, HEADS, I_PER_CORE))),
            "fj": fj,
            "vq": vq,
            "nvq": -vq,
            "adjT": np.ascontiguousarray(adjT[:, sl]),
        })
    return in_maps


def kernel(x, adj, W, a):
    global _BASS, LAST_RESULT
    if _BASS is None:
        _BASS = _build()
    in_maps = _host_prep(x, adj, W, a)
    res = run_bass_kernel_spmd(_BASS, in_maps, core_ids=list(range(CORES)))
    LAST_RESULT = res
    outs = []
    for c in range(CORES):
        nd = res.results[c]["out"]                 # [65, H, 512]
        o2 = res.results[c]["out2"]                # [128(2h x 64f), 2, 512]
        od = res.results[c]["outd"]                # [H, 512]
        num = nd[:F_OUT].copy()                    # [64, H, 512]
        den = nd[F_OUT] + od                       # [H, 512]
        for h in range(HEADS):
            num[:, h, :] += o2[64 * (h % 2):64 * (h % 2) + 64, h // 2, :]
        o = (num / den[None, :, :]).sum(axis=1).T  # [512, 64]
        outs.append(o.astype(np.float32))
    return np.ascontiguousarray(np.concatenate(outs, axis=0))


# revision 4
# speedup vs baseline: 2.3314x; 2.3314x over previous
"""GATv2 layer on 8 Trainium2 NeuronCores (Bass/Tile SPMD).

Identity: exp(lrelu(z)) = max(exp z, exp 0.2z); dividing the softmax by
exp(0.2 ei) gives the unnormalized masked score
    pm[j,i] = adj[j,i] * max(Ti[i]*F[j], V[j])
with Ti = exp(0.8 ei), F = exp(ej), V = exp(0.2 ej).

Aggregation per head: acc = sum_j [0.25*h | 1][j] * pm[j,i]  (PE, [65,512]).

The elementwise pm production is split across DVE and ACT so both engines
work concurrently (chunk pairs, interleaved D/A blocks):
  D pair:  q  = max(tib*F, V)       tensor_scalar 4x   (DVE)
           pm = q * adj             tensor_tensor 2x, batched (DVE)
  A pair:  aq = relu(tib*F - V)     ACT (free per-partition scale/bias)
           m1 = aq * adj            tensor_tensor 2x, batched (DVE)
           + extra PE chain  acc += rhs2^T @ adj  with rhs2 = V*[0.25h | 1]
           (so pm = m1 + V*adj is accumulated in two parts)

Host precomputes h, Ti, F, V, rhs, rhs2 (all O(N*F^2)); the device does all
O(N^2) work.  Core c owns destination rows i in [512c, 512c+512); adj is fed
pre-transposed+sliced per core, fp16, so scores build directly in
[j_partition, i_free] layout for the PE aggregation matmul.
"""

import contextlib
import os
import sys

import numpy as np

for _p in ("/opt/trn_rl_repo", "/root/.axon_site/_ro/trn_rl_repo"):
    if os.path.isdir(_p) and _p not in sys.path:
        sys.path.append(_p)

import concourse.bass as bass
import concourse.mybir as mybir
from concourse import bacc
import concourse.tile as tile
from concourse.tile import add_dep_helper
from concourse.bass_utils import run_bass_kernel_spmd

N = 4096
F_IN = 256
HEADS = 4
F_OUT = 64
CORES = 8
I_PER_CORE = N // CORES          # 512
P = 128
NJC = N // P                     # 32 j-chunks
NPAIR = NJC // 2                 # 16 chunk pairs
NIC = I_PER_CORE // P            # 4 i-chunks
ICOL = F_OUT + 1                 # 65: [0.25*h | ones] per head
A_PAIRS = 9                      # chunk pairs routed to the ACT path

F32 = mybir.dt.float32
F16 = mybir.dt.float16

_BASS = None
LAST_RESULT = None


def _pair_kinds():
    """A pairs at odd slots (strict alternation), D at both ends; the 8th A
    (if any) goes to slot 14."""
    kinds = ["D"] * NPAIR
    odd = [1, 3, 5, 7, 9, 11, 13]
    extra = [14, 2, 6, 10, 4, 8, 12, 0]
    slots = (odd + extra)[:A_PAIRS]
    for s in slots:
        kinds[s] = "A"
    return kinds


def _build(reps=1):
    nc = bacc.Bacc()
    rhs_d = nc.dram_tensor("rhs", [P, NJC, HEADS, ICOL], F16, kind="ExternalInput")
    NA = 2 * A_PAIRS
    rhs2_d = nc.dram_tensor("rhs2", [P, max(NA, 1), 2, P], F16,
                            kind="ExternalInput")
    vq4_d = nc.dram_tensor("vq4", [P, max(NA, 1), HEADS], F16,
                           kind="ExternalInput")
    tib_d = nc.dram_tensor("tib", [P, HEADS, I_PER_CORE], F16, kind="ExternalInput")
    fvn_d = nc.dram_tensor("fvn", [P, 3, NJC, HEADS], F32,
                           kind="ExternalInput")
    adjT_d = nc.dram_tensor("adjT", [N, I_PER_CORE], F16, kind="ExternalInput")
    # per-head [num | den] accumulators; the host does divide + head-mean
    out_d = nc.dram_tensor("out", [ICOL, HEADS, I_PER_CORE], F32,
                           kind="ExternalOutput")
    out2_d = nc.dram_tensor("out2", [P, 2, I_PER_CORE], F32,
                            kind="ExternalOutput")
    outd_d = nc.dram_tensor("outd", [HEADS, I_PER_CORE], F32,
                            kind="ExternalOutput")

    RELU = mybir.ActivationFunctionType.Relu
    CPY = mybir.ActivationFunctionType.Copy
    MULT = mybir.AluOpType.mult
    MAX = mybir.AluOpType.max
    ADD = mybir.AluOpType.add

    kinds = _pair_kinds()

    with tile.TileContext(nc) as tc:
        with (
            tc.tile_pool(name="cst", bufs=1) as cst,
            tc.tile_pool(name="adj", bufs=1) as adjp,
            tc.tile_pool(name="qpm", bufs=3) as qpm,
            tc.tile_pool(name="aqp", bufs=4) as aqp,
            tc.tile_pool(name="fin", bufs=1) as fin,
            tc.tile_pool(name="psacc", bufs=1, space="PSUM") as psacc,
            (tc.For_i(0, reps, 1) if reps > 1 else contextlib.nullcontext()),
        ):
            # ---- loads -------------------------------------------------
            # order: small vectors, first adj chunks, rhs (needed by first
            # MMs), then the rest of adj interleaved with rhs2
            fvn = cst.tile([P, 3, NJC, HEADS], F32, tag="fvn")
            nc.sync.dma_start(fvn[:], fvn_d[:])
            fj = fvn[:, 0]
            vq = fvn[:, 1]
            nvq = fvn[:, 2]
            tib = cst.tile([P, HEADS, I_PER_CORE], F16, tag="tib")
            nc.sync.dma_start(tib[:, :1], tib_d[:, :1])
            nc.sync.dma_start(tib[:, 1:], tib_d[:, 1:])
            adj_sb = adjp.tile([P, NJC, I_PER_CORE], F16, tag="adj_sb")
            rhs = cst.tile([P, NJC, HEADS, ICOL], F16, tag="rhs")
            rhs2 = cst.tile([P, max(NA, 1), 2, P], F16, tag="rhs2")
            vq4 = cst.tile([P, max(NA, 1), HEADS], F16, tag="vq4")
            nc.sync.dma_start(vq4[:], vq4_d[:])
            a_slot = {}
            for pi in range(NPAIR):
                if kinds[pi] == "A":
                    for jj in range(2):
                        a_slot[2 * pi + jj] = len(a_slot)
            # interleave: rhs/rhs2 quarters between adj chunk groups so the
            # first pairs' operands all arrive early
            if A_PAIRS:
                nc.sync.dma_start(rhs2[:, :NA // 2], rhs2_d[:, :NA // 2])
            nc.sync.dma_start(rhs[:, :8], rhs_d[:, :8])
            for j in range(6):
                nc.sync.dma_start(adj_sb[:, j, :], adjT_d[j * P:(j + 1) * P, :])
            nc.sync.dma_start(rhs[:, 8:20], rhs_d[:, 8:20])
            for j in range(6, 10):
                nc.sync.dma_start(adj_sb[:, j, :], adjT_d[j * P:(j + 1) * P, :])
            if A_PAIRS:
                nc.sync.dma_start(rhs2[:, NA // 2:], rhs2_d[:, NA // 2:])
            for j in range(10, 14):
                nc.sync.dma_start(adj_sb[:, j, :], adjT_d[j * P:(j + 1) * P, :])
            nc.sync.dma_start(rhs[:, 20:], rhs_d[:, 20:])
            for j in range(14, 18):
                nc.sync.dma_start(adj_sb[:, j, :], adjT_d[j * P:(j + 1) * P, :])
            for j in range(18, NJC):
                nc.sync.dma_start(adj_sb[:, j, :], adjT_d[j * P:(j + 1) * P, :])

            # warm the ACT table set (Relu) at t=0 so the first real
            # activation doesn't pay the ~2.7us table load
            warm = fin.tile([P, 1], F32, tag="warm")
            nc.gpsimd.memset(warm[:], 0.0)
            nc.scalar.activation(warm[:], warm[:], RELU)

            # pre-touch operands per engine so the first DVE/ACT ops need
            # at most one sync wait each (HW encoding limit)
            junk = fin.tile([P, 8], F32, tag="junk")
            pt1 = nc.vector.tensor_copy(junk[:, 0:1], fj[:, 0, 0:1])
            pt2 = nc.vector.tensor_copy(junk[:, 1:2], vq[:, 0, 0:1])
            pt3 = nc.vector.tensor_copy(junk[:, 2:3], nvq[:, 0, 0:1])
            pt4 = nc.vector.tensor_copy(junk[:, 3:4], tib[:, 0, 0:1])
            pretouch = [pt1, pt2, pt3, pt4]
            CPY = mybir.ActivationFunctionType.Copy
            at1 = nc.scalar.activation(junk[:, 4:5], fj[:, 0, 0:1], CPY)
            at2 = nc.scalar.activation(junk[:, 5:6], nvq[:, 0, 0:1], CPY)
            at3 = nc.scalar.activation(junk[:, 6:7], tib[:, 0, 0:1], CPY)
            pretouch_act = [at1, at2, at3]

            acc = [psacc.tile([ICOL, I_PER_CORE], F32, name=f"acc{h}",
                              tag=f"acc{h}") for h in range(HEADS)]
            acc2 = [psacc.tile([P, I_PER_CORE], F32, name=f"acc2{g}",
                               tag=f"acc2{g}") for g in range(2)]
            accd = psacc.tile([HEADS, I_PER_CORE], F32, tag="accd")

            # per-head accumulation-group bookkeeping
            n_mms = {h: 0 for h in range(HEADS)}
            total_mms = NJC

            def mm(h, lhsT, rhs_op):
                first = n_mms[h] == 0
                n_mms[h] += 1
                last = n_mms[h] == total_mms
                nc.tensor.matmul(acc[h][:], lhsT, rhs_op,
                                 start=first, stop=last)

            first_dve = [True]
            first_act = [True]

            def dep(qi):
                if first_dve[0]:
                    first_dve[0] = False
                    for pt in pretouch:
                        add_dep_helper(qi.ins, pt.ins, sync=False,
                                       reason="pretouch order")

            def dep_act(qi):
                if first_act[0]:
                    first_act[0] = False
                    for pt in pretouch_act:
                        add_dep_helper(qi.ins, pt.ins, sync=False,
                                       reason="pretouch order")

            # The adj-chain MMs depend only on DMA — issue them all
            # upfront so the PE has work during the DVE/ACT ramp; their
            # accumulators retire mid-kernel and evacuate early.  Numerators
            # for head pairs (2g, 2g+1) are packed into one M=128 matmul;
            # denominators for all 4 heads into one M=4 matmul.
            a_chunk_list = [2 * pi + jj for pi in range(NPAIR)
                            if kinds[pi] == "A" for jj in range(2)]
            for n_, j in enumerate(a_chunk_list):
                s = a_slot[j]
                st = n_ == 0
                sp = n_ == len(a_chunk_list) - 1
                for g in range(2):
                    nc.tensor.matmul(acc2[g][:], rhs2[:, s, g, :],
                                     adj_sb[:, j, :], start=st, stop=sp)
                nc.tensor.matmul(accd[:], vq4[:, s, :], adj_sb[:, j, :],
                                 start=st, stop=sp)
            if a_chunk_list:
                out2sb = fin.tile([P, 2, I_PER_CORE], F32, tag="out2sb")
                outdsb = fin.tile([HEADS, I_PER_CORE], F32, tag="outdsb")
                for g in range(2):
                    nc.vector.tensor_copy(out2sb[:, g, :], acc2[g][:])
                nc.vector.tensor_copy(outdsb[:], accd[:])
                nc.sync.dma_start(out2_d[:], out2sb[:])
                nc.sync.dma_start(outd_d[:], outdsb[:])
            else:
                out2sb = fin.tile([P, 2, I_PER_CORE], F32, tag="out2sb")
                outdsb = fin.tile([HEADS, I_PER_CORE], F32, tag="outdsb")
                nc.vector.memset(out2sb[:], 0.0)
                nc.vector.memset(outdsb[:], 0.0)
                nc.sync.dma_start(out2_d[:], out2sb[:])
                nc.sync.dma_start(outd_d[:], outdsb[:])

            # software pipeline: A-pair ACT work is issued LOOKAHEAD pairs
            # early so the DVE never stalls at an A-pair's mask multiply
            # waiting on the ACT engine (DVE queues are strict FIFO).
            LOOKAHEAD = 3
            aq_tiles = {}

            def issue_act(pi):
                if pi >= NPAIR or kinds[pi] != "A":
                    return
                c0 = 2 * pi
                aq = aqp.tile([P, 2, HEADS, I_PER_CORE], F16, tag="aq")
                for jj in range(2):
                    j = c0 + jj
                    for h in range(HEADS):
                        qi = nc.scalar.activation(
                            aq[:, jj, h, :], tib[:, h, :], RELU,
                            scale=fj[:, j, h:h + 1],
                            bias=nvq[:, j, h:h + 1])
                        dep_act(qi)
                aq_tiles[pi] = aq

            for pi in range(LOOKAHEAD):
                issue_act(pi)

            for pi in range(NPAIR):
                issue_act(pi + LOOKAHEAD)
                c0 = 2 * pi
                adj_b = adj_sb[:, c0:c0 + 2, :].unsqueeze(2).broadcast_to(
                    [P, 2, HEADS, I_PER_CORE])
                if kinds[pi] == "D" and pi == NPAIR - 1:
                    # final pair: chunk 30 gets a per-chunk TT; chunk 31 goes
                    # per-head so the very last TT->MM bubble is minimal
                    j = c0
                    q4 = qpm.tile([P, HEADS, I_PER_CORE], F16, tag="q4s")
                    pm4 = qpm.tile([P, HEADS, I_PER_CORE], F16, tag="pm4s")
                    for h in range(HEADS):
                        qi = nc.vector.tensor_scalar(
                            q4[:, h, :], tib[:, h, :],
                            fj[:, j, h:h + 1], vq[:, j, h:h + 1],
                            op0=MULT, op1=MAX)
                        dep(qi)
                    adj_c = adj_sb[:, j, :].unsqueeze(1).broadcast_to(
                        [P, HEADS, I_PER_CORE])
                    nc.vector.tensor_tensor(pm4[:], q4[:], adj_c, op=MULT)
                    for h in range(HEADS):
                        mm(h, rhs[:, j, h, :], pm4[:, h, :])
                    j = c0 + 1
                    q4b = qpm.tile([P, HEADS, I_PER_CORE], F16, tag="q4s")
                    pm4b = qpm.tile([P, HEADS, I_PER_CORE], F16, tag="pm4s")
                    for h in range(HEADS):
                        nc.vector.tensor_scalar(
                            q4b[:, h, :], tib[:, h, :],
                            fj[:, j, h:h + 1], vq[:, j, h:h + 1],
                            op0=MULT, op1=MAX)
                    for h in range(HEADS):
                        nc.vector.tensor_tensor(
                            pm4b[:, h, :], q4b[:, h, :], adj_sb[:, j, :],
                            op=MULT)
                        mm(h, rhs[:, j, h, :], pm4b[:, h, :])
                elif kinds[pi] == "D":
                    q4 = qpm.tile([P, 2, HEADS, I_PER_CORE], F16, tag="q4")
                    pm4 = qpm.tile([P, 2, HEADS, I_PER_CORE], F16, tag="pm4")
                    for jj in range(2):
                        j = c0 + jj
                        for h in range(HEADS):
                            qi = nc.vector.tensor_scalar(
                                q4[:, jj, h, :], tib[:, h, :],
                                fj[:, j, h:h + 1], vq[:, j, h:h + 1],
                                op0=MULT, op1=MAX)
                            dep(qi)
                    nc.vector.tensor_tensor(pm4[:], q4[:], adj_b, op=MULT)
                    for jj in range(2):
                        j = c0 + jj
                        for h in range(HEADS):
                            mm(h, rhs[:, j, h, :], pm4[:, jj, h, :])
                else:
                    aq = aq_tiles.pop(pi)
                    m1 = qpm.tile([P, 2, HEADS, I_PER_CORE], F16, tag="m1")
                    nc.vector.tensor_tensor(m1[:], aq[:], adj_b, op=MULT)
                    for jj in range(2):
                        j = c0 + jj
                        for h in range(HEADS):
                            mm(h, rhs[:, j, h, :], m1[:, jj, h, :])

            # finalize: evacuate the four [num | den] accumulators into one
            # tile and ship with a single DMA; the host does divide +
            # head-mean (O(N*F) numpy)
            numt = fin.tile([ICOL, HEADS, I_PER_CORE], F32, tag="numt")
            nc.scalar.copy(numt[:, 0, :], acc[0][:])
            nc.vector.tensor_copy(numt[:, 1, :], acc[1][:])
            nc.sync.dma_start(out_d[:, :2], numt[:, :2])
            nc.scalar.copy(numt[:, 2, :], acc[2][:])
            nc.vector.tensor_copy(numt[:, 3, :], acc[3][:])
            nc.sync.dma_start(out_d[:, 2:], numt[:, 2:])

    nc.finalize()
    return nc


def _host_prep(x, adj, W, a):
    x = np.asarray(x, np.float32)
    adj = np.asarray(adj)
    W = np.asarray(W, np.float32)
    a = np.asarray(a, np.float32).reshape(-1)
    a1, a2 = a[:F_OUT], a[F_OUT:]

    h = (x @ W).reshape(N, HEADS, F_OUT)          # [N, H, F] f32
    ei = np.einsum("nhf,f->nh", h, a1)            # [N, H]
    ej = np.einsum("nhf,f->nh", h, a2)            # [N, H]

    ti16 = np.exp(0.8 * ei).astype(np.float16)    # [N, H]
    fjv = np.exp(ej).astype(np.float32)           # Sj*Vq = exp(ej)
    vqv = np.exp(0.2 * ej).astype(np.float32)

    hq = (0.25 * h).astype(np.float32)
    rhs = np.empty((P, NJC, HEADS, ICOL), np.float32)
    rhs[:, :, :, :F_OUT] = hq.reshape(NJC, P, HEADS, F_OUT).transpose(1, 0, 2, 3)
    rhs[:, :, :, F_OUT] = 1.0
    rhs2f = rhs * vqv.reshape(NJC, P, HEADS, 1).transpose(1, 0, 2, 3)
    kinds = _pair_kinds()
    a_chunks = [2 * pi + jj for pi in range(NPAIR) if kinds[pi] == "A"
                for jj in range(2)]
    na = max(len(a_chunks), 1)
    rhs2 = np.zeros((P, na, 2, P), np.float32)
    vq4 = np.zeros((P, na, HEADS), np.float32)
    for s, j in enumerate(a_chunks):
        for g in range(2):
            rhs2[:, s, g, :F_OUT] = rhs2f[:, j, 2 * g, :F_OUT]
            rhs2[:, s, g, F_OUT:2 * F_OUT] = rhs2f[:, j, 2 * g + 1, :F_OUT]
        vq4[:, s, :] = rhs2f[:, j, :, F_OUT]

    def jvec(v):
        return np.ascontiguousarray(v.reshape(NJC, P, HEADS).transpose(1, 0, 2))

    fj = jvec(fjv)
    vq = jvec(vqv)
    adjT = adj.T.astype(np.float16)               # [j, i]

    in_maps = []
    for c in range(CORES):
        sl = slice(c * I_PER_CORE, (c + 1) * I_PER_CORE)
        in_maps.append({
            "rhs": rhs.astype(np.float16),
            "rhs2": rhs2.astype(np.float16),
            "vq4": vq4.astype(np.float16),
            "tib": np.ascontiguousarray(
                np.broadcast_to(ti16[sl].T[None], (P, HEADS, I_PER_CORE))),
            "fvn": np.ascontiguousarray(np.stack([fj, vq, -vq], axis=1)),
            "adjT": np.ascontiguousarray(adjT[:, sl]),
        })
    return in_maps


def kernel(x, adj, W, a):
    global _BASS, LAST_RESULT
    if _BASS is None:
        _BASS = _build()
    in_maps = _host_prep(x, adj, W, a)
    res = run_bass_kernel_spmd(_BASS, in_maps, core_ids=list(range(CORES)))
    LAST_RESULT = res
    outs = []
    for c in range(CORES):
        nd = res.results[c]["out"]                 # [65, H, 512]
        o2 = res.results[c]["out2"]                # [128(2h x 64f), 2, 512]
        od = res.results[c]["outd"]                # [H, 512]
        num = nd[:F_OUT].copy()                    # [64, H, 512]
        den = nd[F_OUT] + od                       # [H, 512]
        for h in range(HEADS):
            num[:, h, :] += o2[64 * (h % 2):64 * (h % 2) + 64, h // 2, :]
        o = (num / den[None, :, :]).sum(axis=1).T  # [512, 64]
        outs.append(o.astype(np.float32))
    return np.ascontiguousarray(np.concatenate(outs, axis=0))
